# revision 22
# baseline (speedup 1.0000x reference)
"""Trainium2 Bass kernel for nn_Ani_layer (dense_cnn).

A 64->64ch 3x3 conv whose weight is built from params x basis, with
per-window mean subtraction folded into the conv weights, a vector-norm
"relu" epilogue (out/norm masked where norm<=b) and mean re-add.

Distribution: 8 shards = (batch b in 0..3) x (H half in 0..1); each core
gets a pre-padded bf16 (64ch, 66, 130) input slab and produces
(64ch, 64, 128) fp32. No collectives (halos materialized host-side).

Per-core device pipeline (v2 — modulo-scheduled):
  - x loaded from HBM ONCE ([64, 66, 130] in 3 chunks, one descriptor
    per partition). The three shifted copies the matmul pairing needs
    (xt upper = x shifted down one row, xb lower = x, xb upper = x
    shifted left one column) are built with big-descriptor local
    sbuf->sbuf DMAs using flat offsets (+PW / +0 / +1 elements); the
    wrap garbage lands in pad columns 128/129 which the output DMA
    skips.
  - 5 conv matmuls per 3-row group over CONTIGUOUS N=390 rhs windows;
    psum rows 0-63 = conv, 64-127 = window means (broadcast columns).
  - Epilogue stages are modulo-scheduled across 2-group batches with a
    fixed per-round issue order per engine so Scalar/Vector/GpSimd/PE
    pipeline instead of ping-ponging:
      PE:     accum(r-3) x2, conv(r) x10
      Scalar: copyA(r-3), rsqrt(r-1), square(r)
      Vector: sqsum-select(r-1), STT(r-2)
      GpSimd: r-dup(r-2), copyB(r-3)
      Sync:   out-DMA(r-3)
  - A few warm-up matmuls on a zeroed tile run during the input-load
    lead-in so the PE HAM clock-gate releases (cold PE = 1.2 GHz
    doubles matmul time) before the first real conv.
  - DMA issue cost is ~600ns/instruction on the issuing queue, so
    input loads are spread across Sync + Scalar HWDGE + GpSimd SWDGE.
"""

import os
import sys
from contextlib import ExitStack

for _p in ("/opt/trn_rl_repo", os.path.expanduser("~/.axon_site/_ro/trn_rl_repo")):
    if os.path.isdir(_p) and _p not in sys.path:
        sys.path.insert(0, _p)

import numpy as np
import ml_dtypes

import concourse.bass as bass
import concourse.bacc as bacc
import concourse.tile as tile
import concourse.dve_ops as dve_ops_mod
from concourse import mybir
from concourse.bass_utils import run_bass_kernel_spmd
from concourse.dve_spec import C0, C1, C2, Spec, Src0, Src1, lower, select, sq
from concourse.dve_spec import _has_src1
from concourse.dve_uop import DveOpSpec

F32 = mybir.dt.float32
BF16 = mybir.dt.bfloat16
ALU = mybir.AluOpType
ACTF = mybir.ActivationFunctionType

B, O, I, KS, H, W = 4, 32, 32, 3, 128, 128
NCH = 2 * I          # 64 input channels
HS = H // 2          # 64 output rows per shard
PH, PW = HS + 2, W + 2   # padded shard: 66 x 130
NG, GR = 16, 4       # output tensor viewed as 16 groups of 4 rows
FD = GR * W          # 512 free dim per group
N_CORES = 8
BIG = 1.0e12         # masked pixels: n2 -> BIG so Rsqrt(BIG) ~ 1e-6 ~ 0
NC390 = 390          # real columns per 3-row group (3 * 130)
WARM_MMS = 8         # PE warm-up matmuls during the input-load lead-in


def _register_dve_op(name, spec):
    for op in dve_ops_mod.OPS:
        if op.name == name:
            return op
    row = dve_ops_mod._CUSTOM_DVE_ROW_BASE + len(dve_ops_mod.OPS)
    assert row < 0x20
    dve_ops_mod._SUB_OPCODE_FOR_NAME[name] = row
    uops = lower(spec, ver="v3")
    sha = DveOpSpec(name=name, opcode=row, uops=uops,
                    rd1_en=_has_src1(spec)).sha("v3")
    op = dve_ops_mod.DveOp(name, spec, subdim=False, uops_sha={"v3": sha})
    dve_ops_mod.OPS.append(op)
    dve_ops_mod.CUSTOM_DVE_SPECS[name] = spec
    return op


def _sqsum_sel_op():
    # x = (conv0 + bias0)^2 + (pre-squared t1); sq() on BOTH inputs hangs
    # the DVE, so in1 arrives already squared. C0 = per-partition bias,
    # C1 = b^2, C2 (imm2) = BIG for masked pixels.
    x = sq(Src0 + C0) + Src1
    body = select(x > C1, x, C2)

    def ref(in0, in1, c0, c1, c2):
        xx = (in0.astype(np.float32) + c0) ** 2 + in1.astype(np.float32)
        return np.where(xx > c1, xx, c2)

    return _register_dve_op("SQB_SEL_ANT", Spec(body=body, reference=ref))


def _act_raw(nc, out, in_, func, bias_ap, scale):
    """Emit InstActivation directly (bass bans Rsqrt; our probe measured the
    reciprocal_sqrt LUT at ~5e-5 max rel err over [1e-4, 1e2])."""
    eng = nc.scalar
    inputs = [eng.lower_ap(in_), eng.lower_ap(bias_ap),
              mybir.ImmediateValue(dtype=mybir.dt.float32, value=scale),
              mybir.ImmediateValue(dtype=mybir.dt.float32, value=0.0)]
    return eng.add_instruction(mybir.InstActivation(
        name=nc.get_next_instruction_name(), func=func,
        ins=inputs, outs=[eng.lower_ap(out)]))


_NC = {}


def _build_nc(b2):
    op_sqsum = _sqsum_sel_op()

    nc = bacc.Bacc("TRN2")
    x_d = nc.declare_dram_parameter("x", [NCH, PH, PW], BF16, isOutput=False)
    wp_d = nc.declare_dram_parameter("wp", [3, 128, 128], BF16, isOutput=False)
    wr_d = nc.declare_dram_parameter("wr", [3, NCH, 128], BF16, isOutput=False)
    id_d = nc.declare_dram_parameter("idm", [NCH, NCH], BF16, isOutput=False)
    cst_d = nc.declare_dram_parameter("cst", [NCH, 2], F32, isOutput=False)
    out_d = nc.declare_dram_parameter("out", [NCH, NG * FD], F32, isOutput=True)
    dbg = os.environ.get("ANT_DBG") == "1"
    if dbg:
        xt_dbg = nc.declare_dram_parameter("xt_dbg", [128, PH + 1, PW], BF16, isOutput=True)
        xb_dbg = nc.declare_dram_parameter("xb_dbg", [128, PH + 1, PW], BF16, isOutput=True)

    with tile.TileContext(nc) as tc, ExitStack() as ctx:
        singles = ctx.enter_context(tc.tile_pool(name="singles", bufs=1))
        psum = ctx.enter_context(tc.tile_pool(name="psum", bufs=4, space="PSUM"))
        ep = ctx.enter_context(tc.tile_pool(name="ep", bufs=4))
        outp = ctx.enter_context(tc.tile_pool(name="outp", bufs=6))

        xt = singles.tile([128, PH + 1, PW], BF16, tag="xt")
        xb = singles.tile([128, PH + 1, PW], BF16, tag="xb")
        wp_s = singles.tile([128, 3, 128], BF16, tag="wp")
        wr_s = singles.tile([128, 3, 128], BF16, tag="wr")
        wrb_s = singles.tile([128, 128], BF16, tag="wrb")
        id_s = singles.tile([NCH, NCH], BF16, tag="idm")
        cst = singles.tile([NCH, 2], F32, tag="cst")
        zb = singles.tile([O, 1], F32, tag="zb")
        wt = singles.tile([128, 512], BF16, tag="wt")
        nc.vector.memset(zb, 0.0)
        # wt feeds only the warm-up matmuls; memset on gpsimd, whose engine
        # preamble finishes earliest, so the warm chain starts ASAP.
        nc.gpsimd.memset(wt, 0.0)

        # HBM x chunks (one descriptor per partition each). conv(0) only
        # needs rows 0..9; later chunks land while early rounds run.
        XCH = [(0, 10), (10, 38), (38, PH)]
        # Local shifted-copy chunks (flat offsets; see module docstring).
        TCH = [(0, 9), (9, 37), (37, HS)]        # xt upper (row shift)
        BCH = [(2, 9), (9, 37), (37, PH)]        # xb lower/upper

        xtf = xt.rearrange("p a b -> p (a b)")
        xbf = xb.rearrange("p a b -> p (a b)")

        inp_mode = os.environ.get("ANT_INPUT", "local")
        if inp_mode in ("hbm", "hbm3"):
            # original 4-stream HBM load path (for bisection)
            nc.sync.dma_start(out=cst, in_=cst_d[:, :])
            nc.sync.dma_start(out=wp_s, in_=wp_d.rearrange("j k m -> k j m"))
            nc.sync.dma_start(out=wrb_s[0:NCH], in_=wr_d[0])
            nc.sync.dma_start(out=wrb_s[NCH:128], in_=wr_d[1])
            nc.sync.dma_start(out=wr_s[0:NCH],
                              in_=wr_d.rearrange("j k m -> k j m"))
            if inp_mode == "hbm3":
                CHUNKS = [(0, 10), (10, 38), (38, 66)]
            else:
                CHUNKS = [(0, 6), (6, 16), (16, 26), (26, 36), (36, 46),
                          (46, 56), (56, 66)]
            for ci, (r0, r1) in enumerate(CHUNKS):
                if ci == 1:
                    nc.sync.dma_start(out=id_s, in_=id_d[:, :])
                r1 = min(PH, r1)
                nc.sync.dma_start(out=xt[0:NCH, r0:r1, :], in_=x_d[:, r0:r1, :])
                r1b = min(HS + 1, r1)
                if r1b > r0:
                    nc.sync.dma_start(out=xt[NCH:128, r0:r1b, :],
                                      in_=x_d[:, r0 + 1:r1b + 1, :])
                r0x = max(2, r0)
                if r1 > r0x:
                    nc.sync.dma_start(out=xb[0:NCH, r0x:r1, :],
                                      in_=x_d[:, r0x:r1, :])
                    nc.sync.dma_start(out=xb[NCH:128, r0x:r1, 0:PW - 1],
                                      in_=x_d[:, r0x:r1, 1:PW])
        else:
            # All four streams straight from HBM (sbuf->sbuf chained copies
            # raced — scattered 1.4% corruption). 3 big chunks per stream,
            # chunk 0 on the scalar HWDGE queue (idle until Square(0)),
            # the rest on sync.
            def xc(eng, ci):
                r0, r1 = XCH[ci]
                eng.dma_start(out=xt[0:NCH, r0:r1, :], in_=x_d[:, r0:r1, :])

            def xtu(eng, ci):
                a, b_ = TCH[ci]
                eng.dma_start(out=xt[NCH:128, a:b_, :],
                              in_=x_d[:, a + 1:b_ + 1, :])

            def xbl(eng, ci):
                a, b_ = BCH[ci]
                eng.dma_start(out=xb[0:NCH, a:b_, :], in_=x_d[:, a:b_, :])

            x_flat = x_d.rearrange("p a b -> p (a b)")

            def xbu(eng, ci):
                a, b_ = BCH[ci]
                eng.dma_start(out=xbf[NCH:128, a * PW:b_ * PW - 1],
                              in_=x_flat[:, a * PW + 1:b_ * PW])

            xc(nc.scalar, 0)
            xbl(nc.sync, 0)
            xbu(nc.sync, 0)
            xtu(nc.scalar, 0)
            nc.sync.dma_start(out=wp_s, in_=wp_d.rearrange("j k m -> k j m"))
            nc.sync.dma_start(out=wrb_s[0:NCH], in_=wr_d[0])
            nc.sync.dma_start(out=wrb_s[NCH:128], in_=wr_d[1])
            nc.scalar.dma_start(out=wr_s[0:NCH],
                                in_=wr_d.rearrange("j k m -> k j m"))
            nc.scalar.dma_start(out=cst, in_=cst_d[:, :])
            xc(nc.sync, 1)
            nc.scalar.dma_start(out=id_s, in_=id_d[:, :])
            xtu(nc.sync, 1)
            xbl(nc.sync, 1)
            xbu(nc.sync, 1)
            xc(nc.sync, 2)
            xtu(nc.sync, 2)
            xbl(nc.sync, 2)
            xbu(nc.sync, 2)

        # PE warm-up: garbage matmuls on the zeroed wt tile keep the PE
        # busy from t~0 so the HAM clock-gate releases before conv(0).
        wpt = psum.tile([128, 2, 512], F32, tag="pt")
        for i in range(WARM_MMS):
            nc.tensor.matmul(wpt[:, i % 2, 0:512], wt[:, 0:128], wt,
                             start=True, stop=True)

        groups = [(3 * i, 3) for i in range(21)] + [(63, 1)]
        batches = [(groups[2 * i], groups[2 * i + 1])
                   for i in range(len(groups) // 2)]
        NB = len(batches)

        pts, sq1s, n2ms, r64s, m64s, ots = {}, {}, {}, {}, {}, {}

        def conv(bi):
            pt = psum.tile([128, 2, 512], F32, tag="pt")
            for k, (h0, nr) in enumerate(batches[bi]):
                N = nr * PW
                for j in range(3):
                    nc.tensor.matmul(pt[:, k, 0:N], wp_s[:, j, :],
                                     xtf[:, h0 * PW + j:h0 * PW + j + N],
                                     start=(j == 0), stop=False)
                nc.tensor.matmul(pt[:, k, 0:N], wrb_s,
                                 xbf[:, (h0 + 2) * PW:(h0 + 2) * PW + N],
                                 start=False, stop=False)
                nc.tensor.matmul(pt[:, k, 0:N], wr_s[0:NCH, 2, :],
                                 xtf[0:NCH,
                                     (h0 + 2) * PW + 2:(h0 + 2) * PW + 2 + N],
                                 start=False, stop=True)
            pts[bi] = pt

        def p_accum(bi):
            pt = pts[bi]
            for k, (h0, nr) in enumerate(batches[bi]):
                N = nr * PW
                nc.tensor.matmul(pt[NCH:128, k, 0:N], id_s,
                                 m64s[bi][:, k, 0:N], start=False,
                                 stop=True, tile_position=(0, 64))

        def s_sq(bi):
            sq1 = ep.tile([O, 2, NC390], BF16, tag="sq1")
            nc.scalar.activation(sq1, pts[bi][O:NCH, :, 0:NC390], ACTF.Square,
                                 bias=cst[O:NCH, 0:1], scale=1.0)
            sq1s[bi] = sq1

        def v_custom(bi):
            n2m = ep.tile([O, 2, NC390], BF16, tag="n2m")
            nc.vector._custom_dve(op_sqsum, out=n2m,
                                  in0=pts[bi][0:O, :, 0:NC390],
                                  in1=sq1s[bi].rearrange("p a b -> p (a b)"),
                                  s0=cst[0:O, 0:1], s1=b2, imm2=BIG)
            n2ms[bi] = n2m

        def s_rsq(bi):
            r64 = ep.tile([NCH, 2, NC390], BF16, tag="r64")
            _act_raw(nc, r64[0:O], n2ms[bi], ACTF.Rsqrt, zb, 1.0)
            r64s[bi] = r64

        def g_dup(bi):
            eng = nc.gpsimd if os.environ.get("ANT_DUP", "v") == "gp" \
                else nc.vector
            eng.tensor_copy(r64s[bi][O:NCH], r64s[bi][0:O])

        def v_stt(bi):
            m64 = ep.tile([NCH, 2, NC390], BF16, tag="m64")
            nc.vector.scalar_tensor_tensor(m64, pts[bi][0:NCH, :, 0:NC390],
                                           cst[0:NCH, 0:1], r64s[bi],
                                           ALU.add, ALU.mult)
            m64s[bi] = m64

        def s_cpA(bi):
            ot = outp.tile([NCH, 2, NC390], F32, tag="ot")
            nc.scalar.activation(ot, pts[bi][NCH:128, :, 0:NC390], ACTF.Copy)
            ots[bi] = ot

        def v_cpB(bi):
            pass

        def dma_out(bi):
            (h0a, nra), (h0b, nrb) = batches[bi]
            ot = ots[bi]
            if nra == nrb:
                src = ot[:, :, 0:nra * PW].rearrange(
                    "p a (r c) -> p a r c", c=PW)[:, :, :, 0:W]
                dst = out_d[:, h0a * W:(h0a + 2 * nra) * W].rearrange(
                    "p (a r c) -> p a r c", a=2, c=W)
                nc.sync.dma_start(out=dst, in_=src)
            else:
                for k, (h0, nr) in enumerate(batches[bi]):
                    src = ot[:, k, 0:nr * PW].rearrange(
                        "p (r c) -> p r c", c=PW)[:, :, 0:W]
                    nc.sync.dma_start(
                        out=out_d[:, h0 * W:(h0 + nr) * W].rearrange(
                            "p (r c) -> p r c", c=W),
                        in_=src)

        if os.environ.get("ANT_SCHED", "modulo") == "baseline":
            # baseline-style: conv 3 batches ahead, sequential epilogue
            for bi in range(3):
                conv(bi)
            for bi in range(NB):
                if bi + 3 < NB:
                    conv(bi + 3)
                s_sq(bi)
                v_custom(bi)
                s_rsq(bi)
                g_dup(bi)
                v_stt(bi)
                p_accum(bi)
                s_cpA(bi)
                v_cpB(bi)
                dma_out(bi)
        else:
            # Modulo-scheduled main loop; per-engine issue order per round
            # is load-bearing (in-order queues).
            for r in range(NB + 3):
                if 0 <= r - 3 < NB:
                    p_accum(r - 3)
                if r < NB:
                    conv(r)
                if 0 <= r - 3 < NB:
                    s_cpA(r - 3)
                if 0 <= r - 2 < NB:
                    g_dup(r - 2)
                if 0 <= r - 3 < NB:
                    v_cpB(r - 3)
                if 0 <= r - 3 < NB:
                    dma_out(r - 3)
                if 0 <= r - 1 < NB:
                    v_custom(r - 1)
                if 0 <= r - 1 < NB:
                    s_rsq(r - 1)
                if 0 <= r - 2 < NB:
                    v_stt(r - 2)
                if r < NB:
                    s_sq(r)

        if dbg:
            nc.sync.dma_start(out=xt_dbg[:, :, :], in_=xt[:, :, :])
            nc.sync.dma_start(out=xb_dbg[:, :, :], in_=xb[:, :, :])

    nc.compile()
    return nc


def _get_nc(b2):
    key = float(b2)
    if key not in _NC:
        _NC[key] = _build_nc(key)
    return _NC[key]


def _prep(params, basis, bias_term, b):
    params = np.asarray(params, np.float32)
    basis = np.asarray(basis, np.float32)
    Kr = np.einsum("abcd,cdefgh->abefgh", params, basis)  # (O,I,K,K,2,2)
    kern = Kr.transpose(0, 4, 1, 5, 2, 3).reshape(2 * O, 2 * I, KS, KS)
    # reference pairs patch (kh=q, kw=p) with kern[o2, c, p, q]:
    Wtap = kern.transpose(0, 1, 3, 2)  # [o2, c, dh, dw]
    # fold per-window mean subtraction into the weights
    Ksum = np.stack([Wtap[:, 0::2].sum(axis=(1, 2, 3)),
                     Wtap[:, 1::2].sum(axis=(1, 2, 3))], axis=1)  # [o2, 2]
    cpar = np.arange(NCH) % 2
    Wp = Wtap - (Ksum[:, cpar] / float(I * KS * KS))[:, :, None, None]
    # device output order: dev channel = 32*v + o  <->  torch channel 2*o + v
    perm = np.array([2 * (i % O) + i // O for i in range(NCH)])
    Wdev = np.zeros((128, NCH, KS, KS), np.float32)
    Wdev[0:NCH] = Wp[perm]
    avg_w = np.zeros((NCH, NCH, KS, KS), np.float32)
    for v in (0, 1):
        avg_w[O * v:O * v + O, v::2, :, :] = 1.0 / float(I * KS * KS)
    Wdev[NCH:128] = avg_w
    wp = np.zeros((3, 128, 128), np.float32)
    wr = np.zeros((3, NCH, 128), np.float32)
    for j in range(3):
        wp[j, 0:NCH, :] = Wdev[:, :, 0, j].T
        wp[j, NCH:128, :] = Wdev[:, :, 1, j].T
        wr[j, :, :] = Wdev[:, :, 2, j].T
    bt = np.asarray(bias_term, np.float32).reshape(O, 2)
    cst = np.zeros((NCH, 2), np.float32)
    for v in (0, 1):
        cst[O * v:O * v + O, 0] = bt[:, v]
    cst[0:O, 1] = bt[:, 1]
    b2 = float(np.asarray(b).reshape(-1)[0]) ** 2
    return (wp.astype(ml_dtypes.bfloat16), wr.astype(ml_dtypes.bfloat16),
            cst, b2, perm)


def _run(inputs, trace=False):
    xx = np.asarray(inputs["xx"], np.float32)
    wp, wr, cst, b2, perm = _prep(inputs["params"], inputs["basis"],
                                  inputs["bias_term"], inputs["b"])
    xp = np.pad(xx, ((0, 0), (0, 0), (1, 1), (1, 1)), mode="edge")
    xpb = xp.astype(ml_dtypes.bfloat16)
    idm = np.eye(NCH, dtype=ml_dtypes.bfloat16)
    in_maps = []
    for core in range(N_CORES):
        bb, half = core // 2, core % 2
        shard = np.ascontiguousarray(xpb[bb, :, half * HS:half * HS + PH, :])
        in_maps.append({"x": shard, "wp": wp, "wr": wr, "idm": idm,
                        "cst": cst})
    nc = _get_nc(b2)
    res = run_bass_kernel_spmd(nc, in_maps, list(range(N_CORES)), trace=trace)
    out = np.zeros((B, NCH, H, W), np.float32)
    for core in range(N_CORES):
        bb, half = core // 2, core % 2
        dev = np.asarray(res.results[core]["out"]).reshape(NCH, HS, W)
        out[bb, perm, half * HS:(half + 1) * HS, :] = dev
    return out, res.exec_time_ns


def kernel(**inputs):
    out, _ = _run(inputs, trace=False)
    return out


# revision 23
# speedup vs baseline: 1.1627x; 1.1627x over previous
"""Trainium2 Bass kernel for nn_Ani_layer (dense_cnn).

A 64->64ch 3x3 conv whose weight is built from params x basis, with
per-window mean subtraction folded into the conv weights, a vector-norm
"relu" epilogue (out/norm masked where norm<=b) and mean re-add.

Distribution: 8 shards = (batch b in 0..3) x (H half in 0..1); each core
gets a pre-padded bf16 (64ch, 66, 130) input slab and produces
(64ch, 64, 128) fp32. No collectives (halos materialized host-side).

Per-core device pipeline (v2 — modulo-scheduled):
  - x loaded from HBM ONCE ([64, 66, 130] in 3 chunks, one descriptor
    per partition). The three shifted copies the matmul pairing needs
    (xt upper = x shifted down one row, xb lower = x, xb upper = x
    shifted left one column) are built with big-descriptor local
    sbuf->sbuf DMAs using flat offsets (+PW / +0 / +1 elements); the
    wrap garbage lands in pad columns 128/129 which the output DMA
    skips.
  - 5 conv matmuls per 3-row group over CONTIGUOUS N=390 rhs windows;
    psum rows 0-63 = conv, 64-127 = window means (broadcast columns).
  - Epilogue stages are modulo-scheduled across 2-group batches with a
    fixed per-round issue order per engine so Scalar/Vector/GpSimd/PE
    pipeline instead of ping-ponging:
      PE:     accum(r-3) x2, conv(r) x10
      Scalar: copyA(r-3), rsqrt(r-1), square(r)
      Vector: sqsum-select(r-1), STT(r-2)
      GpSimd: r-dup(r-2), copyB(r-3)
      Sync:   out-DMA(r-3)
  - A few warm-up matmuls on a zeroed tile run during the input-load
    lead-in so the PE HAM clock-gate releases (cold PE = 1.2 GHz
    doubles matmul time) before the first real conv.
  - DMA issue cost is ~600ns/instruction on the issuing queue, so
    input loads are spread across Sync + Scalar HWDGE + GpSimd SWDGE.
"""

import os
import sys
from contextlib import ExitStack

for _p in ("/opt/trn_rl_repo", os.path.expanduser("~/.axon_site/_ro/trn_rl_repo")):
    if os.path.isdir(_p) and _p not in sys.path:
        sys.path.insert(0, _p)

import numpy as np
import ml_dtypes

import concourse.bass as bass
import concourse.bacc as bacc
import concourse.tile as tile
import concourse.dve_ops as dve_ops_mod
from concourse import mybir
from concourse.bass_utils import run_bass_kernel_spmd
from concourse.dve_spec import C0, C1, C2, Spec, Src0, Src1, lower, select, sq
from concourse.dve_spec import _has_src1
from concourse.dve_uop import DveOpSpec

F32 = mybir.dt.float32
BF16 = mybir.dt.bfloat16
ALU = mybir.AluOpType
ACTF = mybir.ActivationFunctionType

B, O, I, KS, H, W = 4, 32, 32, 3, 128, 128
NCH = 2 * I          # 64 input channels
HS = H // 2          # 64 output rows per shard
PH, PW = HS + 2, W + 2   # padded shard: 66 x 130
NG, GR = 16, 4       # output tensor viewed as 16 groups of 4 rows
FD = GR * W          # 512 free dim per group
N_CORES = 8
BIG = 1.0e12         # masked pixels: n2 -> BIG so Rsqrt(BIG) ~ 1e-6 ~ 0
NC390 = 390          # real columns per 3-row group (3 * 130)
WARM_MMS = 8         # PE warm-up matmuls during the input-load lead-in


def _register_dve_op(name, spec):
    for op in dve_ops_mod.OPS:
        if op.name == name:
            return op
    row = dve_ops_mod._CUSTOM_DVE_ROW_BASE + len(dve_ops_mod.OPS)
    assert row < 0x20
    dve_ops_mod._SUB_OPCODE_FOR_NAME[name] = row
    uops = lower(spec, ver="v3")
    sha = DveOpSpec(name=name, opcode=row, uops=uops,
                    rd1_en=_has_src1(spec)).sha("v3")
    op = dve_ops_mod.DveOp(name, spec, subdim=False, uops_sha={"v3": sha})
    dve_ops_mod.OPS.append(op)
    dve_ops_mod.CUSTOM_DVE_SPECS[name] = spec
    return op


def _sqsum_sel_op():
    # x = (conv0 + bias0)^2 + (pre-squared t1); sq() on BOTH inputs hangs
    # the DVE, so in1 arrives already squared. C0 = per-partition bias,
    # C1 = b^2, C2 (imm2) = BIG for masked pixels.
    x = sq(Src0 + C0) + Src1
    body = select(x > C1, x, C2)

    def ref(in0, in1, c0, c1, c2):
        xx = (in0.astype(np.float32) + c0) ** 2 + in1.astype(np.float32)
        return np.where(xx > c1, xx, c2)

    return _register_dve_op("SQB_SEL_ANT", Spec(body=body, reference=ref))


def _act_raw(nc, out, in_, func, bias_ap, scale):
    """Emit InstActivation directly (bass bans Rsqrt; our probe measured the
    reciprocal_sqrt LUT at ~5e-5 max rel err over [1e-4, 1e2])."""
    eng = nc.scalar
    inputs = [eng.lower_ap(in_), eng.lower_ap(bias_ap),
              mybir.ImmediateValue(dtype=mybir.dt.float32, value=scale),
              mybir.ImmediateValue(dtype=mybir.dt.float32, value=0.0)]
    return eng.add_instruction(mybir.InstActivation(
        name=nc.get_next_instruction_name(), func=func,
        ins=inputs, outs=[eng.lower_ap(out)]))


_NC = {}


def _build_nc(b2):
    op_sqsum = _sqsum_sel_op()

    nc = bacc.Bacc("TRN2")
    x_d = nc.declare_dram_parameter("x", [NCH, PH, PW], BF16, isOutput=False)
    wp_d = nc.declare_dram_parameter("wp", [3, 128, 128], BF16, isOutput=False)
    wr_d = nc.declare_dram_parameter("wr", [3, NCH, 128], BF16, isOutput=False)
    id_d = nc.declare_dram_parameter("idm", [NCH, NCH], BF16, isOutput=False)
    cst_d = nc.declare_dram_parameter("cst", [NCH, 2], F32, isOutput=False)
    out_d = nc.declare_dram_parameter("out", [NCH, NG * FD], F32, isOutput=True)
    dbg = os.environ.get("ANT_DBG") == "1"
    if dbg:
        xt_dbg = nc.declare_dram_parameter("xt_dbg", [128, PH + 1, PW], BF16, isOutput=True)
        xb_dbg = nc.declare_dram_parameter("xb_dbg", [128, PH + 1, PW], BF16, isOutput=True)

    with tile.TileContext(nc) as tc, ExitStack() as ctx:
        singles = ctx.enter_context(tc.tile_pool(name="singles", bufs=1))
        psum = ctx.enter_context(tc.tile_pool(name="psum", bufs=4, space="PSUM"))
        ep = ctx.enter_context(tc.tile_pool(name="ep", bufs=4))
        outp = ctx.enter_context(tc.tile_pool(name="outp", bufs=6))

        xt = singles.tile([128, PH + 1, PW], BF16, tag="xt")
        xb = singles.tile([128, PH + 1, PW], BF16, tag="xb")
        wp_s = singles.tile([128, 3, 128], BF16, tag="wp")
        wr_s = singles.tile([128, 3, 128], BF16, tag="wr")
        wrb_s = singles.tile([128, 128], BF16, tag="wrb")
        id_s = singles.tile([NCH, NCH], BF16, tag="idm")
        cst = singles.tile([NCH, 2], F32, tag="cst")
        zb = singles.tile([O, 1], F32, tag="zb")
        wt = singles.tile([128, 512], BF16, tag="wt")
        nc.vector.memset(zb, 0.0)
        # wt feeds only the warm-up matmuls; memset on gpsimd, whose engine
        # preamble finishes earliest, so the warm chain starts ASAP.
        nc.gpsimd.memset(wt, 0.0)

        # HBM x chunks (one descriptor per partition each). conv(0) only
        # needs rows 0..9; later chunks land while early rounds run.
        XCH = [(0, 10), (10, 38), (38, PH)]
        # Local shifted-copy chunks (flat offsets; see module docstring).
        TCH = [(0, 9), (9, 37), (37, HS)]        # xt upper (row shift)
        BCH = [(2, 9), (9, 37), (37, PH)]        # xb lower/upper

        xtf = xt.rearrange("p a b -> p (a b)")
        xbf = xb.rearrange("p a b -> p (a b)")

        inp_mode = os.environ.get("ANT_INPUT", "local")
        if inp_mode in ("hbm", "hbm3"):
            # original 4-stream HBM load path (for bisection)
            nc.sync.dma_start(out=cst, in_=cst_d[:, :])
            nc.sync.dma_start(out=wp_s, in_=wp_d.rearrange("j k m -> k j m"))
            nc.sync.dma_start(out=wrb_s[0:NCH], in_=wr_d[0])
            nc.sync.dma_start(out=wrb_s[NCH:128], in_=wr_d[1])
            nc.sync.dma_start(out=wr_s[0:NCH],
                              in_=wr_d.rearrange("j k m -> k j m"))
            if inp_mode == "hbm3":
                CHUNKS = [(0, 10), (10, 38), (38, 66)]
            else:
                CHUNKS = [(0, 6), (6, 16), (16, 26), (26, 36), (36, 46),
                          (46, 56), (56, 66)]
            for ci, (r0, r1) in enumerate(CHUNKS):
                if ci == 1:
                    nc.sync.dma_start(out=id_s, in_=id_d[:, :])
                r1 = min(PH, r1)
                nc.sync.dma_start(out=xt[0:NCH, r0:r1, :], in_=x_d[:, r0:r1, :])
                r1b = min(HS + 1, r1)
                if r1b > r0:
                    nc.sync.dma_start(out=xt[NCH:128, r0:r1b, :],
                                      in_=x_d[:, r0 + 1:r1b + 1, :])
                r0x = max(2, r0)
                if r1 > r0x:
                    nc.sync.dma_start(out=xb[0:NCH, r0x:r1, :],
                                      in_=x_d[:, r0x:r1, :])
                    nc.sync.dma_start(out=xb[NCH:128, r0x:r1, 0:PW - 1],
                                      in_=x_d[:, r0x:r1, 1:PW])
        else:
            # All four streams straight from HBM (sbuf->sbuf chained copies
            # raced — scattered 1.4% corruption). 3 big chunks per stream,
            # chunk 0 on the scalar HWDGE queue (idle until Square(0)),
            # the rest on sync.
            def xc(eng, ci):
                r0, r1 = XCH[ci]
                eng.dma_start(out=xt[0:NCH, r0:r1, :], in_=x_d[:, r0:r1, :])

            def xtu(eng, ci):
                a, b_ = TCH[ci]
                eng.dma_start(out=xt[NCH:128, a:b_, :],
                              in_=x_d[:, a + 1:b_ + 1, :])

            def xbl(eng, ci):
                a, b_ = BCH[ci]
                eng.dma_start(out=xb[0:NCH, a:b_, :], in_=x_d[:, a:b_, :])

            x_flat = x_d.rearrange("p a b -> p (a b)")

            def xbu(eng, ci):
                a, b_ = BCH[ci]
                eng.dma_start(out=xbf[NCH:128, a * PW:b_ * PW - 1],
                              in_=x_flat[:, a * PW + 1:b_ * PW])

            xc(nc.scalar, 0)
            xbl(nc.sync, 0)
            xbu(nc.sync, 0)
            xtu(nc.scalar, 0)
            nc.sync.dma_start(out=wp_s, in_=wp_d.rearrange("j k m -> k j m"))
            nc.sync.dma_start(out=wrb_s[0:NCH], in_=wr_d[0])
            nc.sync.dma_start(out=wrb_s[NCH:128], in_=wr_d[1])
            nc.scalar.dma_start(out=wr_s[0:NCH],
                                in_=wr_d.rearrange("j k m -> k j m"))
            nc.scalar.dma_start(out=cst, in_=cst_d[:, :])
            xc(nc.sync, 1)
            nc.scalar.dma_start(out=id_s, in_=id_d[:, :])
            xtu(nc.sync, 1)
            xbl(nc.sync, 1)
            xbu(nc.sync, 1)
            xc(nc.sync, 2)
            xtu(nc.sync, 2)
            xbl(nc.sync, 2)
            xbu(nc.sync, 2)

        # PE warm-up: garbage matmuls on the zeroed wt tile keep the PE
        # busy from t~0 so the HAM clock-gate releases before conv(0).
        wpt = psum.tile([128, 2, 512], F32, tag="pt")
        for i in range(WARM_MMS):
            nc.tensor.matmul(wpt[:, i % 2, 0:512], wt[:, 0:128], wt,
                             start=True, stop=True)

        groups = [(3 * i, 3) for i in range(21)] + [(63, 1)]
        batches = [(groups[2 * i], groups[2 * i + 1])
                   for i in range(len(groups) // 2)]
        NB = len(batches)

        pts, sq1s, n2ms, r64s, m64s, ots = {}, {}, {}, {}, {}, {}

        def conv(bi, accum_bi=None):
            pt = psum.tile([128, 2, 512], F32, tag="pt")
            for k, (h0, nr) in enumerate(batches[bi]):
                if k == 1 and accum_bi is not None:
                    p_accum(accum_bi)
                N = nr * PW
                for j in range(3):
                    nc.tensor.matmul(pt[:, k, 0:N], wp_s[:, j, :],
                                     xtf[:, h0 * PW + j:h0 * PW + j + N],
                                     start=(j == 0), stop=False)
                nc.tensor.matmul(pt[:, k, 0:N], wrb_s,
                                 xbf[:, (h0 + 2) * PW:(h0 + 2) * PW + N],
                                 start=False, stop=False)
                nc.tensor.matmul(pt[:, k, 0:N], wr_s[0:NCH, 2, :],
                                 xtf[0:NCH,
                                     (h0 + 2) * PW + 2:(h0 + 2) * PW + 2 + N],
                                 start=False, stop=True)
            pts[bi] = pt

        def p_accum(bi):
            pt = pts[bi]
            for k, (h0, nr) in enumerate(batches[bi]):
                N = nr * PW
                nc.tensor.matmul(pt[NCH:128, k, 0:N], id_s,
                                 m64s[bi][:, k, 0:N], start=False,
                                 stop=True, tile_position=(0, 64))

        def s_sq(bi):
            sq1 = ep.tile([O, 2, NC390], BF16, tag="sq1")
            nc.scalar.activation(sq1, pts[bi][O:NCH, :, 0:NC390], ACTF.Square,
                                 bias=cst[O:NCH, 0:1], scale=1.0)
            sq1s[bi] = sq1

        def v_custom(bi):
            n2m = ep.tile([O, 2, NC390], BF16, tag="n2m")
            nc.vector._custom_dve(op_sqsum, out=n2m,
                                  in0=pts[bi][0:O, :, 0:NC390],
                                  in1=sq1s[bi].rearrange("p a b -> p (a b)"),
                                  s0=cst[0:O, 0:1], s1=b2, imm2=BIG)
            n2ms[bi] = n2m

        def s_rsq(bi):
            r64 = ep.tile([NCH, 2, NC390], BF16, tag="r64")
            _act_raw(nc, r64[0:O], n2ms[bi], ACTF.Rsqrt, zb, 1.0)
            r64s[bi] = r64

        def g_dup(bi):
            eng = nc.gpsimd if os.environ.get("ANT_DUP", "v") == "gp" \
                else nc.vector
            eng.tensor_copy(r64s[bi][O:NCH], r64s[bi][0:O])

        def v_stt(bi):
            m64 = ep.tile([NCH, 2, NC390], BF16, tag="m64")
            nc.vector.scalar_tensor_tensor(m64, pts[bi][0:NCH, :, 0:NC390],
                                           cst[0:NCH, 0:1], r64s[bi],
                                           ALU.add, ALU.mult)
            m64s[bi] = m64

        def s_cpA(bi):
            ot = outp.tile([NCH, 2, NC390], F32, tag="ot")
            nc.scalar.activation(ot, pts[bi][NCH:128, :, 0:NC390], ACTF.Copy)
            ots[bi] = ot

        def v_cpB(bi):
            pass

        def dma_out(bi):
            (h0a, nra), (h0b, nrb) = batches[bi]
            ot = ots[bi]
            if nra == nrb:
                src = ot[:, :, 0:nra * PW].rearrange(
                    "p a (r c) -> p a r c", c=PW)[:, :, :, 0:W]
                dst = out_d[:, h0a * W:(h0a + 2 * nra) * W].rearrange(
                    "p (a r c) -> p a r c", a=2, c=W)
                nc.sync.dma_start(out=dst, in_=src)
            else:
                for k, (h0, nr) in enumerate(batches[bi]):
                    src = ot[:, k, 0:nr * PW].rearrange(
                        "p (r c) -> p r c", c=PW)[:, :, 0:W]
                    nc.sync.dma_start(
                        out=out_d[:, h0 * W:(h0 + nr) * W].rearrange(
                            "p (r c) -> p r c", c=W),
                        in_=src)

        if os.environ.get("ANT_SCHED", "modulo") == "baseline":
            # baseline-style: conv 3 batches ahead, sequential epilogue
            for bi in range(3):
                conv(bi)
            for bi in range(NB):
                if bi + 3 < NB:
                    conv(bi + 3)
                s_sq(bi)
                v_custom(bi)
                s_rsq(bi)
                g_dup(bi)
                v_stt(bi)
                p_accum(bi)
                s_cpA(bi)
                v_cpB(bi)
                dma_out(bi)
        else:
            # Modulo-scheduled main loop; per-engine issue order per round
            # is load-bearing (in-order queues).
            for r in range(NB + 3):
                acc = r - 3 if 0 <= r - 3 < NB else None
                if r < NB:
                    conv(r, acc)
                elif acc is not None:
                    p_accum(acc)
                if 0 <= r - 3 < NB:
                    s_cpA(r - 3)
                if 0 <= r - 2 < NB:
                    g_dup(r - 2)
                if 0 <= r - 3 < NB:
                    v_cpB(r - 3)
                if 0 <= r - 3 < NB:
                    dma_out(r - 3)
                if 0 <= r - 1 < NB:
                    v_custom(r - 1)
                if 0 <= r - 1 < NB:
                    s_rsq(r - 1)
                if 0 <= r - 2 < NB:
                    v_stt(r - 2)
                if r < NB:
                    s_sq(r)

        if dbg:
            nc.sync.dma_start(out=xt_dbg[:, :, :], in_=xt[:, :, :])
            nc.sync.dma_start(out=xb_dbg[:, :, :], in_=xb[:, :, :])

    nc.compile()
    return nc


def _get_nc(b2):
    key = float(b2)
    if key not in _NC:
        _NC[key] = _build_nc(key)
    return _NC[key]


def _prep(params, basis, bias_term, b):
    params = np.asarray(params, np.float32)
    basis = np.asarray(basis, np.float32)
    Kr = np.einsum("abcd,cdefgh->abefgh", params, basis)  # (O,I,K,K,2,2)
    kern = Kr.transpose(0, 4, 1, 5, 2, 3).reshape(2 * O, 2 * I, KS, KS)
    # reference pairs patch (kh=q, kw=p) with kern[o2, c, p, q]:
    Wtap = kern.transpose(0, 1, 3, 2)  # [o2, c, dh, dw]
    # fold per-window mean subtraction into the weights
    Ksum = np.stack([Wtap[:, 0::2].sum(axis=(1, 2, 3)),
                     Wtap[:, 1::2].sum(axis=(1, 2, 3))], axis=1)  # [o2, 2]
    cpar = np.arange(NCH) % 2
    Wp = Wtap - (Ksum[:, cpar] / float(I * KS * KS))[:, :, None, None]
    # device output order: dev channel = 32*v + o  <->  torch channel 2*o + v
    perm = np.array([2 * (i % O) + i // O for i in range(NCH)])
    Wdev = np.zeros((128, NCH, KS, KS), np.float32)
    Wdev[0:NCH] = Wp[perm]
    avg_w = np.zeros((NCH, NCH, KS, KS), np.float32)
    for v in (0, 1):
        avg_w[O * v:O * v + O, v::2, :, :] = 1.0 / float(I * KS * KS)
    Wdev[NCH:128] = avg_w
    wp = np.zeros((3, 128, 128), np.float32)
    wr = np.zeros((3, NCH, 128), np.float32)
    for j in range(3):
        wp[j, 0:NCH, :] = Wdev[:, :, 0, j].T
        wp[j, NCH:128, :] = Wdev[:, :, 1, j].T
        wr[j, :, :] = Wdev[:, :, 2, j].T
    bt = np.asarray(bias_term, np.float32).reshape(O, 2)
    cst = np.zeros((NCH, 2), np.float32)
    for v in (0, 1):
        cst[O * v:O * v + O, 0] = bt[:, v]
    cst[0:O, 1] = bt[:, 1]
    b2 = float(np.asarray(b).reshape(-1)[0]) ** 2
    return (wp.astype(ml_dtypes.bfloat16), wr.astype(ml_dtypes.bfloat16),
            cst, b2, perm)


def _run(inputs, trace=False):
    xx = np.asarray(inputs["xx"], np.float32)
    wp, wr, cst, b2, perm = _prep(inputs["params"], inputs["basis"],
                                  inputs["bias_term"], inputs["b"])
    xp = np.pad(xx, ((0, 0), (0, 0), (1, 1), (1, 1)), mode="edge")
    xpb = xp.astype(ml_dtypes.bfloat16)
    idm = np.eye(NCH, dtype=ml_dtypes.bfloat16)
    in_maps = []
    for core in range(N_CORES):
        bb, half = core // 2, core % 2
        shard = np.ascontiguousarray(xpb[bb, :, half * HS:half * HS + PH, :])
        in_maps.append({"x": shard, "wp": wp, "wr": wr, "idm": idm,
                        "cst": cst})
    nc = _get_nc(b2)
    res = run_bass_kernel_spmd(nc, in_maps, list(range(N_CORES)), trace=trace)
    out = np.zeros((B, NCH, H, W), np.float32)
    for core in range(N_CORES):
        bb, half = core // 2, core % 2
        dev = np.asarray(res.results[core]["out"]).reshape(NCH, HS, W)
        out[bb, perm, half * HS:(half + 1) * HS, :] = dev
    return out, res.exec_time_ns


def kernel(**inputs):
    out, _ = _run(inputs, trace=False)
    return out
